# revision 15
# baseline (speedup 1.0000x reference)
"""Bahdanau-style causal additive attention on 8 TRN2 NeuronCores.

Math (per batch b):
    Qp = Q @ Wq.T ; Kp = K @ Wk.T                     [T, H]
    scores[t, s] = sum_h v[h] * tanh(Qp[t, h] + Kp[s, h])
    causal mask (s <= t), softmax over s -> alpha     [T, T]
    ctx = alpha @ K                                   [T, H]
Returns (ctx, alpha).

Sharding: B=2, T=512 -> 1024 query rows total. Causal structure means a
query row t only needs key columns s in [0, 128*ceil((t+1)/128)). Queries
are grouped by their s-tile count k in {1,2,3,4}; each core gets 32 queries
of every k (perfect load balance, identical instruction stream across
cores -> true SPMD). Core c (b = c//4, r = c%4) owns, for each k, queries
t = 128*(k-1) + 32*r + j, j in 0..31, processed in sub-batches of 16.

Per-core engine pipeline per sub-batch (16 queries, width w = 128k):
  DVE : E[h, i, :w] = Kp[h, :w] + Qp[h, qi]     (tensor_scalar add, 32x)
  ACT : tanh in place over [128, 16*w] (one instruction per H-half)
  PE  : scores rows via sliding one-hot v weight -> PSUM [16, w]
  DVE/ACT: causal mask add, -max, exp(x-max), sum, 1/sum, normalize
  PE  : alpha^T via transpose, ctx = alpha @ K
"""

import numpy as np

import concourse.bass as bass
import concourse.bacc as bacc
import concourse.mybir as mybir
from concourse.bass_utils import run_bass_kernel_spmd
from concourse.tile import TileContext

B, T, H = 2, 512, 256
P = 128
NCORES = 8
SUB = 16  # queries per sub-batch
NEG = -1.0e9
F32 = mybir.dt.float32


def core_query_ts(c: int) -> list[int]:
    """Query t-indices owned by core c, in device processing order."""
    r = c % 4
    ts = []
    for k in range(1, 5):
        for sb in range(2):
            for i in range(SUB):
                ts.append(128 * (k - 1) + 32 * r + 16 * sb + i)
    return ts


def build_program(reps: int = 1, loop_iters: int = 0) -> bass.Bass:
    nc = bacc.Bacc()

    qt_d = nc.dram_tensor("qt", [H, 128], F32, kind="ExternalInput")
    kt_d = nc.dram_tensor("kt", [H, T], F32, kind="ExternalInput")
    kb_d = nc.dram_tensor("kb", [T, H], F32, kind="ExternalInput")
    wqt_d = nc.dram_tensor("wqt", [H, H], F32, kind="ExternalInput")
    wkt_d = nc.dram_tensor("wkt", [H, H], F32, kind="ExternalInput")
    g_d = nc.dram_tensor("g", [2, P, 2 * SUB - 1], F32, kind="ExternalInput")
    mask_d = nc.dram_tensor("mask", [2, SUB, P], F32, kind="ExternalInput")
    id_d = nc.dram_tensor("ident", [P, P], F32, kind="ExternalInput")

    alpha_d = nc.dram_tensor("alpha_part", [128, T], F32, kind="ExternalOutput")
    ctx_d = nc.dram_tensor("ctx_part", [128, H], F32, kind="ExternalOutput")

    AF = mybir.ActivationFunctionType
    AX = mybir.AxisListType

    with TileContext(nc) as tc:
        with (
            tc.tile_pool(name="const", bufs=1) as cp,
            tc.tile_pool(name="proj", bufs=1) as pp,
            tc.tile_pool(name="psproj", bufs=1, space=bass.MemorySpace.PSUM) as psp,
            tc.tile_pool(name="epool", bufs=2) as ep,
            tc.tile_pool(name="pscore", bufs=2, space=bass.MemorySpace.PSUM) as pss,
            tc.tile_pool(name="soft", bufs=2) as sp,
            tc.tile_pool(name="pstr", bufs=2, space=bass.MemorySpace.PSUM) as pst,
            tc.tile_pool(name="psctx", bufs=2, space=bass.MemorySpace.PSUM) as psc,
        ):
            # ---- constants / inputs to SBUF
            wqt = [cp.tile([P, H], F32, tag=f"wqt{a}", name=f"wqt{a}") for a in range(2)]
            wkt = [cp.tile([P, H], F32, tag=f"wkt{a}", name=f"wkt{a}") for a in range(2)]
            qts = [cp.tile([P, 128], F32, tag=f"qt{a}", name=f"qt{a}") for a in range(2)]
            kts = [cp.tile([P, T], F32, tag=f"kt{a}", name=f"kt{a}") for a in range(2)]
            kbs = [cp.tile([P, H], F32, tag=f"kb{s}", name=f"kb{s}") for s in range(4)]
            gs = [cp.tile([P, 2 * SUB - 1], F32, tag=f"g{a}", name=f"g{a}") for a in range(2)]
            masks = [cp.tile([SUB, P], F32, tag=f"mask{sb}", name=f"mask{sb}") for sb in range(2)]
            ident = cp.tile([P, P], F32, tag="ident", name="ident")

            for a in range(2):
                nc.sync.dma_start(out=wqt[a], in_=wqt_d[a * P : (a + 1) * P, :])
                nc.sync.dma_start(out=qts[a], in_=qt_d[a * P : (a + 1) * P, :])
                nc.sync.dma_start(out=wkt[a], in_=wkt_d[a * P : (a + 1) * P, :])
                nc.sync.dma_start(out=kts[a], in_=kt_d[a * P : (a + 1) * P, :])
                nc.sync.dma_start(out=gs[a], in_=g_d[a])
                nc.sync.dma_start(out=masks[a], in_=mask_d[a])
            for s in range(4):
                nc.sync.dma_start(out=kbs[s], in_=kb_d[s * P : (s + 1) * P, :])
            nc.sync.dma_start(out=ident, in_=id_d[:, :])

            # ---- projections: QpT[a] = (Wq @ Q_sel.T) rows [128a:128a+128]
            qpt = [pp.tile([P, 128], F32, tag=f"qpt{a}", name=f"qpt{a}") for a in range(2)]
            kpt = [pp.tile([P, T], F32, tag=f"kpt{a}", name=f"kpt{a}") for a in range(2)]
            for a in range(2):
                psq = psp.tile([P, 128], F32, tag="psq", name="psq")
                for bp in range(2):
                    nc.tensor.matmul(
                        psq,
                        wqt[bp][:, a * P : (a + 1) * P],
                        qts[bp],
                        start=(bp == 0),
                        stop=(bp == 1),
                    )
                nc.vector.tensor_copy(qpt[a], psq)
            for a in range(2):
                psk = psp.tile([P, T], F32, tag="psk", name="psk")
                for bp in range(2):
                    nc.tensor.matmul(
                        psk,
                        wkt[bp][:, a * P : (a + 1) * P],
                        kts[bp],
                        start=(bp == 0),
                        stop=(bp == 1),
                    )
                nc.vector.tensor_copy(kpt[a], psk)

            # ---- main loop over k-groups and sub-batches
            if loop_iters:
                with tc.For_i(
                    0,
                    loop_iters,
                    1,
                    hint_engines=(
                        mybir.EngineType.DVE,
                        mybir.EngineType.PE,
                        mybir.EngineType.Activation,
                    ),
                ):
                    _main_loop(nc, ep, pss, sp, pst, psc, qpt, kpt, gs, masks,
                               ident, kbs, alpha_d, ctx_d)
            else:
                for _rep in range(reps):
                    _main_loop(nc, ep, pss, sp, pst, psc, qpt, kpt, gs, masks,
                               ident, kbs, alpha_d, ctx_d)

    nc.compile()
    return nc


def _main_loop(nc, ep, pss, sp, pst, psc, qpt, kpt, gs, masks, ident, kbs,
               alpha_d, ctx_d):
    AF = mybir.ActivationFunctionType
    AX = mybir.AxisListType
    if True:
            for k in range(1, 5):
                w = 128 * k
                for sb in range(2):
                    qb = 32 * (k - 1) + SUB * sb  # first query slot of sub-batch

                    # broadcast add: E[h, a, i, s] = Kp[h, s] + Qp[h, qb+i]
                    e = ep.tile([P, 2, SUB, w], F32, tag="e", name="e")
                    for i in range(SUB):
                        for a in range(2):
                            nc.vector.tensor_scalar_add(
                                e[:, a, i, :],
                                kpt[a][:, :w],
                                qpt[a][:, qb + i : qb + i + 1],
                            )
                    for a in range(2):
                        nc.scalar.activation(e[:, a], e[:, a], AF.Tanh)

                    # scores[i, s] = sum_h v[h] * E[h, a, i, s]
                    ps_s = pss.tile([SUB, T], F32, tag="scores", name="scores")
                    for i in range(SUB):
                        for a in range(2):
                            nc.tensor.matmul(
                                ps_s[:, :w],
                                gs[a][:, SUB - 1 - i : 2 * SUB - 1 - i],
                                e[:, a, i, :],
                                start=(i == 0 and a == 0),
                                stop=(i == SUB - 1 and a == 1),
                            )

                    # causal mask on the diagonal s-tile
                    nc.vector.tensor_add(
                        ps_s[:, w - P : w], ps_s[:, w - P : w], masks[sb]
                    )

                    # softmax over s (free dim)
                    negmax = sp.tile([SUB, 1], F32, tag="negmax", name="negmax")
                    nc.vector.reduce_max(negmax, ps_s[:, :w], axis=AX.X, negate=True)
                    exps = sp.tile([SUB, T], F32, tag="expS", name="expS")
                    nc.scalar.activation(
                        exps[:, :w], ps_s[:, :w], AF.Exp, bias=negmax
                    )
                    ssum = sp.tile([SUB, 1], F32, tag="ssum", name="ssum")
                    nc.vector.reduce_sum(ssum, exps[:, :w], axis=AX.X)
                    rsum = sp.tile([SUB, 1], F32, tag="rsum", name="rsum")
                    nc.vector.reciprocal(rsum, ssum)

                    alpha = sp.tile([SUB, T], F32, tag="alpha", name="alpha")
                    if w < T:
                        nc.gpsimd.memset(alpha[:, w:], 0.0)
                    nc.vector.tensor_scalar_mul(alpha[:, :w], exps[:, :w], rsum)
                    nc.sync.dma_start(
                        out=alpha_d[qb : qb + SUB, :], in_=alpha
                    )

                    # ctx = alpha @ K_b : transpose alpha tiles, then matmul
                    alphat = sp.tile([P, 4, SUB], F32, tag="alphat", name="alphat")
                    for s in range(k):
                        ps_t = pst.tile([P, SUB], F32, tag="tr", name="tr")
                        nc.tensor.transpose(
                            ps_t,
                            alpha[:, s * P : (s + 1) * P],
                            ident[:SUB, :SUB],
                        )
                        nc.vector.tensor_copy(alphat[:, s, :], ps_t)
                    ps_c = psc.tile([SUB, H], F32, tag="ctx", name="ctx")
                    for s in range(k):
                        nc.tensor.matmul(
                            ps_c,
                            alphat[:, s, :],
                            kbs[s],
                            start=(s == 0),
                            stop=(s == k - 1),
                        )
                    ctxs = sp.tile([SUB, H], F32, tag="ctxs", name="ctxs")
                    nc.vector.tensor_copy(ctxs, ps_c)
                    nc.sync.dma_start(out=ctx_d[qb : qb + SUB, :], in_=ctxs)


def make_in_maps(Q, K, Wq, Wk, v):
    """Per-core input dicts. All host-side numpy prep."""
    Q = np.ascontiguousarray(np.asarray(Q, dtype=np.float32))
    K = np.ascontiguousarray(np.asarray(K, dtype=np.float32))
    Wq = np.asarray(Wq, dtype=np.float32)
    Wk = np.asarray(Wk, dtype=np.float32)
    v = np.asarray(v, dtype=np.float32)

    wqt = np.ascontiguousarray(Wq.T)
    wkt = np.ascontiguousarray(Wk.T)
    g = np.zeros((2, P, 2 * SUB - 1), dtype=np.float32)
    g[0, :, SUB - 1] = v[:P]
    g[1, :, SUB - 1] = v[P:]
    ident = np.eye(P, dtype=np.float32)

    in_maps = []
    for c in range(NCORES):
        b, r = c // 4, c % 4
        ts = core_query_ts(c)
        mask = np.zeros((2, SUB, P), dtype=np.float32)
        for sb in range(2):
            for i in range(SUB):
                lim = 32 * r + 16 * sb + i  # allow s_loc <= lim
                mask[sb, i, lim + 1 :] = NEG
        in_maps.append(
            {
                "qt": np.ascontiguousarray(Q[b][ts].T),
                "kt": np.ascontiguousarray(K[b].T),
                "kb": K[b],
                "wqt": wqt,
                "wkt": wkt,
                "g": g,
                "mask": mask,
                "ident": ident,
            }
        )
    return in_maps


_NC_CACHE: dict = {}


def get_program(reps: int = 1, loop_iters: int = 0) -> bass.Bass:
    key = f"nc{reps}_{loop_iters}"
    if key not in _NC_CACHE:
        _NC_CACHE[key] = build_program(reps, loop_iters)
    return _NC_CACHE[key]


def get_runner(reps: int = 1, loop_iters: int = 0):
    """Compiled multi-core PJRT callable, cached. Returns (call, out_names).

    call(in_maps) -> list of per-core {name: np.ndarray}. Mirrors the
    multi-core branch of bass2jax.run_bass_via_pjrt but reuses one jitted
    function so repeat invocations skip JAX retracing.
    """
    key = f"run{reps}_{loop_iters}"
    if key in _NC_CACHE:
        return _NC_CACHE[key]
    import jax
    import numpy as _np
    from jax.experimental.shard_map import shard_map
    from jax.sharding import Mesh, PartitionSpec
    from concourse import bass2jax

    bass2jax.install_neuronx_cc_hook()
    nc = get_program(reps, loop_iters)
    assert nc.dbg_addr is None
    pname = nc.partition_id_tensor.name if nc.partition_id_tensor else None

    in_names, out_names, out_avals = [], [], []
    for alloc in nc.m.functions[0].allocations:
        if not isinstance(alloc, mybir.MemoryLocationSet):
            continue
        name = alloc.memorylocations[0].name
        if alloc.kind == "ExternalInput":
            if name != pname:
                in_names.append(name)
        elif alloc.kind == "ExternalOutput":
            out_names.append(name)
            out_avals.append(
                jax.core.ShapedArray(
                    tuple(alloc.tensor_shape), mybir.dt.np(alloc.dtype)
                )
            )
    n_params = len(in_names)
    n_outs = len(out_avals)
    all_names = in_names + out_names
    if pname is not None:
        all_names = all_names + [pname]
    all_names = tuple(all_names)

    def _body(*args):
        operands = list(args)
        if pname is not None:
            operands.append(bass2jax.partition_id_tensor())
        outs = bass2jax._bass_exec_p.bind(
            *operands,
            out_avals=tuple(out_avals),
            in_names=all_names,
            out_names=tuple(out_names),
            lowering_input_output_aliases=(),
            sim_require_finite=True,
            sim_require_nnan=True,
            nc=nc,
        )
        return tuple(outs)

    devices = jax.devices()[:NCORES]
    mesh = Mesh(_np.asarray(devices), ("core",))
    in_specs = (PartitionSpec("core"),) * (n_params + n_outs)
    out_specs = (PartitionSpec("core"),) * n_outs
    donate = tuple(range(n_params, n_params + n_outs))
    sharded = jax.jit(
        shard_map(
            _body, mesh=mesh, in_specs=in_specs, out_specs=out_specs,
            check_rep=False,
        ),
        donate_argnums=donate,
        keep_unused=True,
    )

    def call(in_maps):
        concat_in = [
            _np.concatenate([_np.asarray(m[name]) for m in in_maps], axis=0)
            for name in in_names
        ]
        concat_zeros = [
            _np.zeros((NCORES * a.shape[0], *a.shape[1:]), a.dtype)
            for a in out_avals
        ]
        out_arrs = sharded(*concat_in, *concat_zeros)
        out_arrs = [_np.asarray(o) for o in out_arrs]
        return [
            {
                name: out_arrs[i].reshape(NCORES, *out_avals[i].shape)[c]
                for i, name in enumerate(out_names)
            }
            for c in range(NCORES)
        ]

    _NC_CACHE[key] = call
    return call


def run_spmd(Q, K, Wq, Wk, v, **kwargs):
    nc = get_program()
    in_maps = make_in_maps(Q, K, Wq, Wk, v)
    return run_bass_kernel_spmd(nc, in_maps, core_ids=list(range(NCORES)), **kwargs)


def assemble(results):
    ctx = np.zeros((B, T, H), dtype=np.float32)
    alpha = np.zeros((B, T, T), dtype=np.float32)
    for c in range(NCORES):
        b = c // 4
        ts = core_query_ts(c)
        ctx[b, ts, :] = results[c]["ctx_part"]
        alpha[b, ts, :] = results[c]["alpha_part"]
    return ctx, alpha


def kernel(Q, K, Wq, Wk, v):
    call = get_runner()
    results = call(make_in_maps(Q, K, Wq, Wk, v))
    return assemble(results)


# revision 38
# speedup vs baseline: 3.0773x; 3.0773x over previous
"""Bahdanau-style causal additive attention on 8 TRN2 NeuronCores.

Math (per batch b):
    Qp = Q @ Wq.T ; Kp = K @ Wk.T                     [T, H]
    scores[t, s] = sum_h v[h] * tanh(Qp[t, h] + Kp[s, h])
    causal mask (s <= t), softmax over s -> alpha     [T, T]
    ctx = alpha @ K                                   [T, H]
Returns (ctx, alpha).

Sharding: B=2, T=512 -> 1024 query rows. Causality means query t only needs
key columns s < 64*ceil((t+1)/64). Queries are grouped by that 64-tile
count j in {1..8}; each core gets 16 queries of every j (load balanced,
identical instruction stream across cores -> SPMD). Core c (b = c//4,
r = c%4) owns, for each j, queries t = 64*(j-1) + 16*r + i, i in 0..15.

Per-core pipeline per sub-batch (16 queries, width w = 64j):
  DVE : E[h, i, :w] = Kp[h, :w] + Qp[h, qi]   (tensor_scalar, bf16 4x mode)
  ACT : tanh in place, chunked for pipelining
  PE  : score rows via sliding one-hot v weight (bf16) -> PSUM fp32 [16, w]
  DVE/ACT: causal mask add, -max, exp(x-max), sum, 1/sum, normalize
  PE  : alpha^T via PE transpose, ctx = alpha @ K (bf16)
"""

import numpy as np

import concourse.bass as bass
import concourse.bacc as bacc
import concourse.mybir as mybir
from concourse.bass_utils import run_bass_kernel_spmd
from concourse.tile import TileContext

B, T, H = 2, 512, 256
P = 128
NCORES = 8
SUB = 16  # queries per sub-batch
NJ = 8  # number of causal width groups (width = 64j)
NEG = -1.0e9
F32 = mybir.dt.float32
BF16 = mybir.dt.bfloat16
E_BF16 = True  # score/ctx matmul operands in bf16 (PE streams bf16 4x faster)
EDT = BF16 if E_BF16 else F32


def core_query_ts(c: int) -> list[int]:
    """Query t-indices owned by core c, in device processing order."""
    r = c % 4
    return [
        64 * (j - 1) + 16 * r + i for j in range(1, NJ + 1) for i in range(SUB)
    ]


def build_program(reps: int = 1, loop_iters: int = 0, stage: str = "full") -> bass.Bass:
    nc = bacc.Bacc()

    qt_d = nc.dram_tensor("qt", [H, 128], F32, kind="ExternalInput")
    kt_d = nc.dram_tensor("kt", [H, T], F32, kind="ExternalInput")
    kb_d = nc.dram_tensor("kb", [T, H], EDT, kind="ExternalInput")
    wqt_d = nc.dram_tensor("wqt", [H, H], F32, kind="ExternalInput")
    wkt_d = nc.dram_tensor("wkt", [H, H], F32, kind="ExternalInput")
    g_d = nc.dram_tensor("g", [2, P, 2 * SUB - 1], EDT, kind="ExternalInput")
    mask_d = nc.dram_tensor("mask", [SUB, 64], F32, kind="ExternalInput")
    id_d = nc.dram_tensor("ident", [P, P], F32, kind="ExternalInput")

    alpha_d = nc.dram_tensor("alpha_part", [128, T], F32, kind="ExternalOutput")
    ctx_d = nc.dram_tensor("ctx_part", [128, H], F32, kind="ExternalOutput")

    with TileContext(nc) as tc:
        with (
            tc.tile_pool(name="const", bufs=1) as cp,
            tc.tile_pool(name="proj", bufs=1) as pp,
            tc.tile_pool(name="epool", bufs=3) as ep,
            tc.tile_pool(name="pscore", bufs=4, space=bass.MemorySpace.PSUM) as pss,
            tc.tile_pool(name="soft", bufs=2) as sp,
            tc.tile_pool(name="pstr", bufs=2, space=bass.MemorySpace.PSUM) as pst,
            tc.tile_pool(name="psctx", bufs=2, space=bass.MemorySpace.PSUM) as psc,
        ):
            # ---- constants / inputs to SBUF
            wqt = [cp.tile([P, H], F32, tag=f"wqt{a}", name=f"wqt{a}") for a in range(2)]
            wkt = [cp.tile([P, H], F32, tag=f"wkt{a}", name=f"wkt{a}") for a in range(2)]
            qts = [cp.tile([P, 128], F32, tag=f"qt{a}", name=f"qt{a}") for a in range(2)]
            kts = [cp.tile([P, T], F32, tag=f"kt{a}", name=f"kt{a}") for a in range(2)]
            kbs = [cp.tile([P, H], EDT, tag=f"kb{s}", name=f"kb{s}") for s in range(4)]
            gs = [cp.tile([P, 2 * SUB - 1], EDT, tag=f"g{a}", name=f"g{a}") for a in range(2)]
            maskt = cp.tile([SUB, 64], F32, tag="mask", name="maskt")
            ident = cp.tile([P, P], F32, tag="ident", name="ident")

            for a in range(2):
                nc.sync.dma_start(out=wqt[a], in_=wqt_d[a * P : (a + 1) * P, :])
                nc.sync.dma_start(out=qts[a], in_=qt_d[a * P : (a + 1) * P, :])
                nc.sync.dma_start(out=wkt[a], in_=wkt_d[a * P : (a + 1) * P, :])
                nc.sync.dma_start(out=kts[a], in_=kt_d[a * P : (a + 1) * P, :])
                nc.sync.dma_start(out=gs[a], in_=g_d[a])
            for s in range(4):
                nc.sync.dma_start(out=kbs[s], in_=kb_d[s * P : (s + 1) * P, :])
            nc.sync.dma_start(out=maskt, in_=mask_d[:, :])
            nc.sync.dma_start(out=ident, in_=id_d[:, :])

            # ---- projections: QpT/KpT = W @ X_sel.T, rows [128a:128a+128]
            qpt = [pp.tile([P, 128], F32, tag=f"qpt{a}", name=f"qpt{a}") for a in range(2)]
            kpt = [pp.tile([P, T], EDT, tag=f"kpt{a}", name=f"kpt{a}") for a in range(2)]
            for a in range(2):
                psq = pss.tile([P, 128], F32, tag="scores", name="psq")
                for bp in range(2):
                    nc.tensor.matmul(
                        psq,
                        wqt[bp][:, a * P : (a + 1) * P],
                        qts[bp],
                        start=(bp == 0),
                        stop=(bp == 1),
                    )
                nc.vector.tensor_copy(qpt[a], psq)
            for a in range(2):
                psk = pss.tile([P, T], F32, tag="scores", name="psk")
                for bp in range(2):
                    nc.tensor.matmul(
                        psk,
                        wkt[bp][:, a * P : (a + 1) * P],
                        kts[bp],
                        start=(bp == 0),
                        stop=(bp == 1),
                    )
                nc.vector.tensor_copy(kpt[a], psk)

            # ---- main loop
            if loop_iters:
                with tc.For_i(
                    0,
                    loop_iters,
                    1,
                    hint_engines=(
                        mybir.EngineType.DVE,
                        mybir.EngineType.PE,
                        mybir.EngineType.Activation,
                    ),
                ):
                    _main_loop(nc, ep, pss, sp, pst, psc, qpt, kpt, gs, maskt,
                               ident, kbs, alpha_d, ctx_d, stage)
            else:
                for _rep in range(reps):
                    _main_loop(nc, ep, pss, sp, pst, psc, qpt, kpt, gs, maskt,
                               ident, kbs, alpha_d, ctx_d, stage)

    nc.compile()
    return nc


def _main_loop(nc, ep, pss, sp, pst, psc, qpt, kpt, gs, maskt, ident, kbs,
               alpha_d, ctx_d, stage="full"):
    AF = mybir.ActivationFunctionType
    AX = mybir.AxisListType

    def front(j):
        """adds + tanh + score matmuls for width group j; returns psum tile."""
        w = 64 * j
        qb = SUB * (j - 1)
        # broadcast add: E[h, a, i, s] = Kp[h, s] + Qp[h, qb+i]
        # a-major so tanh(a=0) only waits on the first 16 adds
        e = ep.tile([P, 2, SUB, w], EDT, tag="e", name="e")
        for a in range(2):
            for i in range(SUB):
                nc.vector.tensor_scalar_add(
                    e[:, a, i, :],
                    kpt[a][:, :w],
                    qpt[a][:, qb + i : qb + i + 1],
                )
            if stage != "adds":
                nc.scalar.activation(e[:, a], e[:, a], AF.Tanh)
        if stage in ("tanh", "adds"):
            return None
        # scores[i, s] = sum_h v[h] * E[h, a, i, s]; a-major so the a=0
        # matmul sweep overlaps the a=1 tanh
        ps_s = pss.tile([SUB, T], F32, tag="scores", name="scores")
        for a in range(2):
            for i in range(SUB):
                nc.tensor.matmul(
                    ps_s[:, :w],
                    gs[a][:, SUB - 1 - i : 2 * SUB - 1 - i],
                    e[:, a, i, :],
                    start=(i == 0 and a == 0),
                    stop=(i == SUB - 1 and a == 1),
                )
        return ps_s

    def tail(j, ps_s):
        """softmax + ctx for width group j (emitted one group late so the
        exp never blocks the next group's tanh in ACT's FIFO)."""
        w = 64 * j
        qb = SUB * (j - 1)
        if stage == "scores":
            junk = sp.tile([SUB, T], F32, tag="junk", name="junk")
            nc.vector.tensor_copy(junk[:, :w], ps_s[:, :w])
            return
        # softmax over s; |scores| <= sum|v| ~ 15 so exp needs no max
        # subtraction. Additive causal mask pre-exp; exp emits the row-sum
        # via accum_out so DVE does no reduce.
        nc.vector.tensor_add(ps_s[:, w - 64 : w], ps_s[:, w - 64 : w], maskt)
        exps = sp.tile([SUB, T], F32, tag="expS", name="expS")
        ssum = sp.tile([SUB, 1], F32, tag="ssum", name="ssum")
        nc.scalar.activation(exps[:, :w], ps_s[:, :w], AF.Exp, accum_out=ssum)
        rsum = sp.tile([SUB, 1], F32, tag="rsum", name="rsum")
        nc.vector.reciprocal(rsum, ssum)

        alpha = sp.tile([SUB, T], F32, tag="alpha", name="alpha")
        if w < T:
            nc.gpsimd.memset(alpha[:, w:], 0.0)
        nc.vector.tensor_scalar_mul(alpha[:, :w], exps[:, :w], rsum)
        nc.sync.dma_start(out=alpha_d[qb : qb + SUB, :], in_=alpha)
        if stage == "softmax":
            return
        # ctx = alpha @ K_b = diag(1/sum) (exps @ K_b): transpose the
        # unnormalized exps in 128-wide chunks (zero-pad the tail for odd
        # j), matmul against 128-row K tiles, scale rows by 1/sum on the
        # way out of PSUM.
        nk = (j + 1) // 2
        if j % 2 == 1:
            nc.gpsimd.memset(exps[:, w : w + 64], 0.0)
        alphat = sp.tile([P, 4, SUB], EDT, tag="alphat", name="alphat")
        for s in range(nk):
            ps_t = pst.tile([P, SUB], F32, tag="tr", name="tr")
            nc.tensor.transpose(
                ps_t, exps[:, s * P : (s + 1) * P], ident[:SUB, :SUB]
            )
            nc.vector.tensor_copy(alphat[:, s, :], ps_t)
        ps_c = psc.tile([SUB, H], F32, tag="ctx", name="ctx")
        for s in range(nk):
            nc.tensor.matmul(
                ps_c,
                alphat[:, s, :],
                kbs[s],
                start=(s == 0),
                stop=(s == nk - 1),
            )
        ctxs = sp.tile([SUB, H], F32, tag="ctxs", name="ctxs")
        nc.vector.tensor_scalar_mul(ctxs, ps_c, rsum)
        nc.sync.dma_start(out=ctx_d[qb : qb + SUB, :], in_=ctxs)

    pending = None
    for j in range(1, NJ + 1):
        ps = front(j)
        if pending is not None:
            tail(pending[0], pending[1])
        pending = (j, ps) if ps is not None else None
    if pending is not None:
        tail(pending[0], pending[1])


def _to_edt(x):
    if E_BF16:
        import ml_dtypes

        return x.astype(ml_dtypes.bfloat16)
    return x


def make_in_maps(Q, K, Wq, Wk, v):
    """Per-core input dicts. All host-side numpy prep."""
    Q = np.ascontiguousarray(np.asarray(Q, dtype=np.float32))
    K = np.ascontiguousarray(np.asarray(K, dtype=np.float32))
    Wq = np.asarray(Wq, dtype=np.float32)
    Wk = np.asarray(Wk, dtype=np.float32)
    v = np.asarray(v, dtype=np.float32)

    wqt = np.ascontiguousarray(Wq.T)
    wkt = np.ascontiguousarray(Wk.T)
    g = np.zeros((2, P, 2 * SUB - 1), dtype=np.float32)
    g[0, :, SUB - 1] = v[:P]
    g[1, :, SUB - 1] = v[P:]
    g = _to_edt(g)
    ident = np.eye(P, dtype=np.float32)
    kbs = [_to_edt(K[b]) for b in range(B)]

    in_maps = []
    for c in range(NCORES):
        b, r = c // 4, c % 4
        ts = core_query_ts(c)
        mask = np.zeros((SUB, 64), dtype=np.float32)
        for i in range(SUB):
            mask[i, 16 * r + i + 1 :] = NEG
        in_maps.append(
            {
                "qt": np.ascontiguousarray(Q[b][ts].T),
                "kt": np.ascontiguousarray(K[b].T),
                "kb": kbs[b],
                "wqt": wqt,
                "wkt": wkt,
                "g": g,
                "mask": mask,
                "ident": ident,
            }
        )
    return in_maps


_NC_CACHE: dict = {}


def get_program(reps: int = 1, loop_iters: int = 0, stage: str = "full") -> bass.Bass:
    key = f"nc{reps}_{loop_iters}_{stage}"
    if key not in _NC_CACHE:
        _NC_CACHE[key] = build_program(reps, loop_iters, stage)
    return _NC_CACHE[key]


def get_runner(reps: int = 1, loop_iters: int = 0, stage: str = "full"):
    """Compiled multi-core PJRT callable, cached. call(in_maps) -> per-core dicts."""
    key = f"run{reps}_{loop_iters}_{stage}"
    if key in _NC_CACHE:
        return _NC_CACHE[key]
    import jax
    import numpy as _np
    from jax.experimental.shard_map import shard_map
    from jax.sharding import Mesh, PartitionSpec
    from concourse import bass2jax

    bass2jax.install_neuronx_cc_hook()
    nc = get_program(reps, loop_iters, stage)
    assert nc.dbg_addr is None
    pname = nc.partition_id_tensor.name if nc.partition_id_tensor else None

    in_names, out_names, out_avals = [], [], []
    for alloc in nc.m.functions[0].allocations:
        if not isinstance(alloc, mybir.MemoryLocationSet):
            continue
        name = alloc.memorylocations[0].name
        if alloc.kind == "ExternalInput":
            if name != pname:
                in_names.append(name)
        elif alloc.kind == "ExternalOutput":
            out_names.append(name)
            out_avals.append(
                jax.core.ShapedArray(
                    tuple(alloc.tensor_shape), mybir.dt.np(alloc.dtype)
                )
            )
    n_params = len(in_names)
    n_outs = len(out_avals)
    all_names = in_names + out_names
    if pname is not None:
        all_names = all_names + [pname]
    all_names = tuple(all_names)

    def _body(*args):
        operands = list(args)
        if pname is not None:
            operands.append(bass2jax.partition_id_tensor())
        outs = bass2jax._bass_exec_p.bind(
            *operands,
            out_avals=tuple(out_avals),
            in_names=all_names,
            out_names=tuple(out_names),
            lowering_input_output_aliases=(),
            sim_require_finite=True,
            sim_require_nnan=True,
            nc=nc,
        )
        return tuple(outs)

    devices = jax.devices()[:NCORES]
    mesh = Mesh(np.asarray(devices), ("core",))
    in_specs = (PartitionSpec("core"),) * (n_params + n_outs)
    out_specs = (PartitionSpec("core"),) * n_outs
    donate = tuple(range(n_params, n_params + n_outs))
    sharded = jax.jit(
        shard_map(
            _body, mesh=mesh, in_specs=in_specs, out_specs=out_specs,
            check_rep=False,
        ),
        donate_argnums=donate,
        keep_unused=True,
    )

    def call(in_maps):
        concat_in = [
            _np.concatenate([_np.asarray(m[name]) for m in in_maps], axis=0)
            for name in in_names
        ]
        concat_zeros = [
            _np.zeros((NCORES * a.shape[0], *a.shape[1:]), a.dtype)
            for a in out_avals
        ]
        out_arrs = sharded(*concat_in, *concat_zeros)
        out_arrs = [_np.asarray(o) for o in out_arrs]
        return [
            {
                name: out_arrs[i].reshape(NCORES, *out_avals[i].shape)[c]
                for i, name in enumerate(out_names)
            }
            for c in range(NCORES)
        ]

    _NC_CACHE[key] = call
    return call


def run_spmd(Q, K, Wq, Wk, v, **kwargs):
    nc = get_program()
    in_maps = make_in_maps(Q, K, Wq, Wk, v)
    return run_bass_kernel_spmd(nc, in_maps, core_ids=list(range(NCORES)), **kwargs)


def assemble(results):
    ctx = np.zeros((B, T, H), dtype=np.float32)
    alpha = np.zeros((B, T, T), dtype=np.float32)
    for c in range(NCORES):
        b = c // 4
        ts = core_query_ts(c)
        ctx[b, ts, :] = results[c]["ctx_part"]
        alpha[b, ts, :] = results[c]["alpha_part"]
    return ctx, alpha


def kernel(Q, K, Wq, Wk, v):
    call = get_runner()
    results = call(make_in_maps(Q, K, Wq, Wk, v))
    return assemble(results)


# revision 40
# speedup vs baseline: 3.3117x; 1.0762x over previous
"""Bahdanau-style causal additive attention on 8 TRN2 NeuronCores.

Math (per batch b):
    Qp = Q @ Wq.T ; Kp = K @ Wk.T                     [T, H]
    scores[t, s] = sum_h v[h] * tanh(Qp[t, h] + Kp[s, h])
    causal mask (s <= t), softmax over s -> alpha     [T, T]
    ctx = alpha @ K                                   [T, H]
Returns (ctx, alpha).

Sharding: B=2, T=512 -> 1024 query rows. Causality means query t only needs
key columns s < 64*ceil((t+1)/64). Queries are grouped by that 64-tile
count j in {1..8}; each core gets 16 queries of every j (load balanced,
identical instruction stream across cores -> SPMD). Core c (b = c//4,
r = c%4) owns, for each j, queries t = 64*(j-1) + 16*r + i, i in 0..15.

Per-core pipeline per sub-batch (16 queries, width w = 64j):
  DVE : E[h, i, :w] = Kp[h, :w] + Qp[h, qi]   (tensor_scalar, bf16 4x mode)
  ACT : tanh in place, chunked for pipelining
  PE  : score rows via sliding one-hot v weight (bf16) -> PSUM fp32 [16, w]
  DVE/ACT: causal mask add, -max, exp(x-max), sum, 1/sum, normalize
  PE  : alpha^T via PE transpose, ctx = alpha @ K (bf16)
"""

import numpy as np

import concourse.bass as bass
import concourse.bacc as bacc
import concourse.mybir as mybir
from concourse.bass_utils import run_bass_kernel_spmd
from concourse.tile import TileContext

B, T, H = 2, 512, 256
P = 128
NCORES = 8
SUB = 16  # queries per sub-batch
NJ = 8  # number of causal width groups (width = 64j)
NEG = -1.0e9
F32 = mybir.dt.float32
BF16 = mybir.dt.bfloat16
E_BF16 = True  # score/ctx matmul operands in bf16 (PE streams bf16 4x faster)
EDT = BF16 if E_BF16 else F32


def core_query_ts(c: int) -> list[int]:
    """Query t-indices owned by core c, in device processing order."""
    r = c % 4
    return [
        64 * (j - 1) + 16 * r + i for j in range(1, NJ + 1) for i in range(SUB)
    ]


def build_program(reps: int = 1, loop_iters: int = 0, stage: str = "full") -> bass.Bass:
    nc = bacc.Bacc()

    qt_d = nc.dram_tensor("qt", [H, 128], F32, kind="ExternalInput")
    kt_d = nc.dram_tensor("kt", [H, T], F32, kind="ExternalInput")
    kb_d = nc.dram_tensor("kb", [T, H], F32, kind="ExternalInput")
    wqt_d = nc.dram_tensor("wqt", [H, H], F32, kind="ExternalInput")
    wkt_d = nc.dram_tensor("wkt", [H, H], F32, kind="ExternalInput")
    g_d = nc.dram_tensor("g", [2, P, 2 * SUB - 1], EDT, kind="ExternalInput")
    mask_d = nc.dram_tensor("mask", [SUB, 64], F32, kind="ExternalInput")
    id_d = nc.dram_tensor("ident", [P, P], F32, kind="ExternalInput")

    alpha_d = nc.dram_tensor("alpha_part", [128, T], F32, kind="ExternalOutput")
    ctx_d = nc.dram_tensor("ctx_part", [128, H], F32, kind="ExternalOutput")

    with TileContext(nc) as tc:
        with (
            tc.tile_pool(name="const", bufs=1) as cp,
            tc.tile_pool(name="proj", bufs=1) as pp,
            tc.tile_pool(name="epool", bufs=3 if E_BF16 else 2) as ep,
            tc.tile_pool(name="pscore", bufs=4, space=bass.MemorySpace.PSUM) as pss,
            tc.tile_pool(name="soft", bufs=2) as sp,
            tc.tile_pool(name="pstr", bufs=2, space=bass.MemorySpace.PSUM) as pst,
            tc.tile_pool(name="psctx", bufs=2, space=bass.MemorySpace.PSUM) as psc,
        ):
            # ---- constants / inputs to SBUF
            wqt = [cp.tile([P, H], F32, tag=f"wqt{a}", name=f"wqt{a}") for a in range(2)]
            wkt = [cp.tile([P, H], F32, tag=f"wkt{a}", name=f"wkt{a}") for a in range(2)]
            qts = [cp.tile([P, 128], F32, tag=f"qt{a}", name=f"qt{a}") for a in range(2)]
            kts = [cp.tile([P, T], F32, tag=f"kt{a}", name=f"kt{a}") for a in range(2)]
            kbs = [cp.tile([P, H], F32, tag=f"kb{s}", name=f"kb{s}") for s in range(4)]
            gs = [cp.tile([P, 2 * SUB - 1], EDT, tag=f"g{a}", name=f"g{a}") for a in range(2)]
            maskt = cp.tile([SUB, 64], F32, tag="mask", name="maskt")
            ident = cp.tile([P, P], F32, tag="ident", name="ident")

            for a in range(2):
                nc.sync.dma_start(out=wqt[a], in_=wqt_d[a * P : (a + 1) * P, :])
                nc.sync.dma_start(out=qts[a], in_=qt_d[a * P : (a + 1) * P, :])
                nc.sync.dma_start(out=wkt[a], in_=wkt_d[a * P : (a + 1) * P, :])
                nc.sync.dma_start(out=kts[a], in_=kt_d[a * P : (a + 1) * P, :])
                nc.sync.dma_start(out=gs[a], in_=g_d[a])
            for s in range(4):
                nc.sync.dma_start(out=kbs[s], in_=kb_d[s * P : (s + 1) * P, :])
            nc.sync.dma_start(out=maskt, in_=mask_d[:, :])
            nc.sync.dma_start(out=ident, in_=id_d[:, :])

            # ---- projections: QpT/KpT = W @ X_sel.T, rows [128a:128a+128]
            qpt = [pp.tile([P, 128], F32, tag=f"qpt{a}", name=f"qpt{a}") for a in range(2)]
            kpt = [pp.tile([P, T], EDT, tag=f"kpt{a}", name=f"kpt{a}") for a in range(2)]
            for a in range(2):
                psq = pss.tile([P, 128], F32, tag="scores", name="psq")
                for bp in range(2):
                    nc.tensor.matmul(
                        psq,
                        wqt[bp][:, a * P : (a + 1) * P],
                        qts[bp],
                        start=(bp == 0),
                        stop=(bp == 1),
                    )
                nc.vector.tensor_copy(qpt[a], psq)
            for a in range(2):
                psk = pss.tile([P, T], F32, tag="scores", name="psk")
                for bp in range(2):
                    nc.tensor.matmul(
                        psk,
                        wkt[bp][:, a * P : (a + 1) * P],
                        kts[bp],
                        start=(bp == 0),
                        stop=(bp == 1),
                    )
                nc.vector.tensor_copy(kpt[a], psk)

            # ---- main loop
            if loop_iters:
                with tc.For_i(
                    0,
                    loop_iters,
                    1,
                    hint_engines=(
                        mybir.EngineType.DVE,
                        mybir.EngineType.PE,
                        mybir.EngineType.Activation,
                    ),
                ):
                    _main_loop(nc, ep, pss, sp, pst, psc, qpt, kpt, gs, maskt,
                               ident, kbs, alpha_d, ctx_d, stage)
            else:
                for _rep in range(reps):
                    _main_loop(nc, ep, pss, sp, pst, psc, qpt, kpt, gs, maskt,
                               ident, kbs, alpha_d, ctx_d, stage)

    nc.compile()
    return nc


def _main_loop(nc, ep, pss, sp, pst, psc, qpt, kpt, gs, maskt, ident, kbs,
               alpha_d, ctx_d, stage="full"):
    AF = mybir.ActivationFunctionType
    AX = mybir.AxisListType

    def front(j):
        """adds + tanh + score matmuls for width group j; returns psum tile."""
        w = 64 * j
        qb = SUB * (j - 1)
        # broadcast add: E[h, a, i, s] = Kp[h, s] + Qp[h, qb+i]
        # a-major so tanh(a=0) only waits on the first 16 adds
        e = ep.tile([P, 2, SUB, w], EDT, tag="e", name="e")
        for a in range(2):
            for i in range(SUB):
                nc.vector.tensor_scalar_add(
                    e[:, a, i, :],
                    kpt[a][:, :w],
                    qpt[a][:, qb + i : qb + i + 1],
                )
            if stage != "adds":
                nc.scalar.activation(e[:, a], e[:, a], AF.Tanh)
        if stage in ("tanh", "adds"):
            return None
        # scores[i, s] = sum_h v[h] * E[h, a, i, s]; a-major so the a=0
        # matmul sweep overlaps the a=1 tanh
        ps_s = pss.tile([SUB, T], F32, tag="scores", name="scores")
        for a in range(2):
            for i in range(SUB):
                nc.tensor.matmul(
                    ps_s[:, :w],
                    gs[a][:, SUB - 1 - i : 2 * SUB - 1 - i],
                    e[:, a, i, :],
                    start=(i == 0 and a == 0),
                    stop=(i == SUB - 1 and a == 1),
                )
        return ps_s

    def tail(j, ps_s):
        """softmax + ctx for width group j (emitted one group late so the
        exp never blocks the next group's tanh in ACT's FIFO)."""
        w = 64 * j
        qb = SUB * (j - 1)
        if stage == "scores":
            junk = sp.tile([SUB, T], F32, tag="junk", name="junk")
            nc.vector.tensor_copy(junk[:, :w], ps_s[:, :w])
            return
        # softmax over s; |scores| <= sum|v| ~ 15 so exp needs no max
        # subtraction. Additive causal mask pre-exp; exp emits the row-sum
        # via accum_out so DVE does no reduce.
        nc.vector.tensor_add(ps_s[:, w - 64 : w], ps_s[:, w - 64 : w], maskt)
        exps = sp.tile([SUB, T], F32, tag="expS", name="expS")
        ssum = sp.tile([SUB, 1], F32, tag="ssum", name="ssum")
        nc.scalar.activation(exps[:, :w], ps_s[:, :w], AF.Exp, accum_out=ssum)
        rsum = sp.tile([SUB, 1], F32, tag="rsum", name="rsum")
        nc.vector.reciprocal(rsum, ssum)

        alpha = sp.tile([SUB, T], F32, tag="alpha", name="alpha")
        if w < T:
            nc.gpsimd.memset(alpha[:, w:], 0.0)
        nc.vector.tensor_scalar_mul(alpha[:, :w], exps[:, :w], rsum)
        nc.sync.dma_start(out=alpha_d[qb : qb + SUB, :], in_=alpha)
        if stage == "softmax":
            return
        # ctx = alpha @ K_b = diag(1/sum) (exps @ K_b): transpose the
        # unnormalized exps in 128-wide chunks (zero-pad the tail for odd
        # j), matmul against 128-row K tiles, scale rows by 1/sum on the
        # way out of PSUM.
        nk = (j + 1) // 2
        if j % 2 == 1:
            nc.gpsimd.memset(exps[:, w : w + 64], 0.0)
        alphat = sp.tile([P, 4, SUB], F32, tag="alphat", name="alphat")
        for s in range(nk):
            ps_t = pst.tile([P, SUB], F32, tag="tr", name="tr")
            nc.tensor.transpose(
                ps_t, exps[:, s * P : (s + 1) * P], ident[:SUB, :SUB]
            )
            nc.vector.tensor_copy(alphat[:, s, :], ps_t)
        ps_c = psc.tile([SUB, H], F32, tag="ctx", name="ctx")
        for s in range(nk):
            nc.tensor.matmul(
                ps_c,
                alphat[:, s, :],
                kbs[s],
                start=(s == 0),
                stop=(s == nk - 1),
            )
        ctxs = sp.tile([SUB, H], F32, tag="ctxs", name="ctxs")
        nc.vector.tensor_scalar_mul(ctxs, ps_c, rsum)
        nc.sync.dma_start(out=ctx_d[qb : qb + SUB, :], in_=ctxs)

    pending = None
    for j in range(1, NJ + 1):
        ps = front(j)
        if pending is not None:
            tail(pending[0], pending[1])
        pending = (j, ps) if ps is not None else None
    if pending is not None:
        tail(pending[0], pending[1])


def _to_edt(x):
    if E_BF16:
        import ml_dtypes

        return x.astype(ml_dtypes.bfloat16)
    return x


def make_in_maps(Q, K, Wq, Wk, v):
    """Per-core input dicts. All host-side numpy prep."""
    Q = np.ascontiguousarray(np.asarray(Q, dtype=np.float32))
    K = np.ascontiguousarray(np.asarray(K, dtype=np.float32))
    Wq = np.asarray(Wq, dtype=np.float32)
    Wk = np.asarray(Wk, dtype=np.float32)
    v = np.asarray(v, dtype=np.float32)

    wqt = np.ascontiguousarray(Wq.T)
    wkt = np.ascontiguousarray(Wk.T)
    g = np.zeros((2, P, 2 * SUB - 1), dtype=np.float32)
    g[0, :, SUB - 1] = v[:P]
    g[1, :, SUB - 1] = v[P:]
    g = _to_edt(g)
    ident = np.eye(P, dtype=np.float32)
    kbs = [K[b] for b in range(B)]

    in_maps = []
    for c in range(NCORES):
        b, r = c // 4, c % 4
        ts = core_query_ts(c)
        mask = np.zeros((SUB, 64), dtype=np.float32)
        for i in range(SUB):
            mask[i, 16 * r + i + 1 :] = NEG
        in_maps.append(
            {
                "qt": np.ascontiguousarray(Q[b][ts].T),
                "kt": np.ascontiguousarray(K[b].T),
                "kb": kbs[b],
                "wqt": wqt,
                "wkt": wkt,
                "g": g,
                "mask": mask,
                "ident": ident,
            }
        )
    return in_maps


_NC_CACHE: dict = {}


def get_program(reps: int = 1, loop_iters: int = 0, stage: str = "full") -> bass.Bass:
    key = f"nc{reps}_{loop_iters}_{stage}"
    if key not in _NC_CACHE:
        _NC_CACHE[key] = build_program(reps, loop_iters, stage)
    return _NC_CACHE[key]


def get_runner(reps: int = 1, loop_iters: int = 0, stage: str = "full"):
    """Compiled multi-core PJRT callable, cached. call(in_maps) -> per-core dicts."""
    key = f"run{reps}_{loop_iters}_{stage}"
    if key in _NC_CACHE:
        return _NC_CACHE[key]
    import jax
    import numpy as _np
    from jax.experimental.shard_map import shard_map
    from jax.sharding import Mesh, PartitionSpec
    from concourse import bass2jax

    bass2jax.install_neuronx_cc_hook()
    nc = get_program(reps, loop_iters, stage)
    assert nc.dbg_addr is None
    pname = nc.partition_id_tensor.name if nc.partition_id_tensor else None

    in_names, out_names, out_avals = [], [], []
    for alloc in nc.m.functions[0].allocations:
        if not isinstance(alloc, mybir.MemoryLocationSet):
            continue
        name = alloc.memorylocations[0].name
        if alloc.kind == "ExternalInput":
            if name != pname:
                in_names.append(name)
        elif alloc.kind == "ExternalOutput":
            out_names.append(name)
            out_avals.append(
                jax.core.ShapedArray(
                    tuple(alloc.tensor_shape), mybir.dt.np(alloc.dtype)
                )
            )
    n_params = len(in_names)
    n_outs = len(out_avals)
    all_names = in_names + out_names
    if pname is not None:
        all_names = all_names + [pname]
    all_names = tuple(all_names)

    def _body(*args):
        operands = list(args)
        if pname is not None:
            operands.append(bass2jax.partition_id_tensor())
        outs = bass2jax._bass_exec_p.bind(
            *operands,
            out_avals=tuple(out_avals),
            in_names=all_names,
            out_names=tuple(out_names),
            lowering_input_output_aliases=(),
            sim_require_finite=True,
            sim_require_nnan=True,
            nc=nc,
        )
        return tuple(outs)

    devices = jax.devices()[:NCORES]
    mesh = Mesh(np.asarray(devices), ("core",))
    in_specs = (PartitionSpec("core"),) * (n_params + n_outs)
    out_specs = (PartitionSpec("core"),) * n_outs
    donate = tuple(range(n_params, n_params + n_outs))
    sharded = jax.jit(
        shard_map(
            _body, mesh=mesh, in_specs=in_specs, out_specs=out_specs,
            check_rep=False,
        ),
        donate_argnums=donate,
        keep_unused=True,
    )

    def call(in_maps):
        concat_in = [
            _np.concatenate([_np.asarray(m[name]) for m in in_maps], axis=0)
            for name in in_names
        ]
        concat_zeros = [
            _np.zeros((NCORES * a.shape[0], *a.shape[1:]), a.dtype)
            for a in out_avals
        ]
        out_arrs = sharded(*concat_in, *concat_zeros)
        out_arrs = [_np.asarray(o) for o in out_arrs]
        return [
            {
                name: out_arrs[i].reshape(NCORES, *out_avals[i].shape)[c]
                for i, name in enumerate(out_names)
            }
            for c in range(NCORES)
        ]

    _NC_CACHE[key] = call
    return call


def run_spmd(Q, K, Wq, Wk, v, **kwargs):
    nc = get_program()
    in_maps = make_in_maps(Q, K, Wq, Wk, v)
    return run_bass_kernel_spmd(nc, in_maps, core_ids=list(range(NCORES)), **kwargs)


def assemble(results):
    ctx = np.zeros((B, T, H), dtype=np.float32)
    alpha = np.zeros((B, T, T), dtype=np.float32)
    for c in range(NCORES):
        b = c // 4
        ts = core_query_ts(c)
        ctx[b, ts, :] = results[c]["ctx_part"]
        alpha[b, ts, :] = results[c]["alpha_part"]
    return ctx, alpha


def kernel(Q, K, Wq, Wk, v):
    call = get_runner()
    results = call(make_in_maps(Q, K, Wq, Wk, v))
    return assemble(results)


# revision 42
# speedup vs baseline: 3.4109x; 1.0299x over previous
"""Bahdanau-style causal additive attention on 8 TRN2 NeuronCores.

Math (per batch b):
    Qp = Q @ Wq.T ; Kp = K @ Wk.T                     [T, H]
    scores[t, s] = sum_h v[h] * tanh(Qp[t, h] + Kp[s, h])
    causal mask (s <= t), softmax over s -> alpha     [T, T]
    ctx = alpha @ K                                   [T, H]
Returns (ctx, alpha).

Sharding: B=2, T=512 -> 1024 query rows. Causality means query t only needs
key columns s < 64*ceil((t+1)/64). Queries are grouped by that 64-tile
count j in {1..8}; each core gets 16 queries of every j (load balanced,
identical instruction stream across cores -> SPMD). Core c (b = c//4,
r = c%4) owns, for each j, queries t = 64*(j-1) + 16*r + i, i in 0..15.

Per-core pipeline per sub-batch (16 queries, width w = 64j):
  DVE : E[h, i, :w] = Kp[h, :w] + Qp[h, qi]   (tensor_scalar, bf16 4x mode)
  ACT : tanh in place, chunked for pipelining
  PE  : score rows via sliding one-hot v weight (bf16) -> PSUM fp32 [16, w]
  DVE/ACT: causal mask add, -max, exp(x-max), sum, 1/sum, normalize
  PE  : alpha^T via PE transpose, ctx = alpha @ K (bf16)
"""

import numpy as np

import concourse.bass as bass
import concourse.bacc as bacc
import concourse.mybir as mybir
from concourse.bass_utils import run_bass_kernel_spmd
from concourse.tile import TileContext

B, T, H = 2, 512, 256
P = 128
NCORES = 8
SUB = 16  # queries per sub-batch
NJ = 8  # number of causal width groups (width = 64j)
NEG = -1.0e9
F32 = mybir.dt.float32
BF16 = mybir.dt.bfloat16
E_BF16 = True  # score/ctx matmul operands in bf16 (PE streams bf16 4x faster)
EDT = BF16 if E_BF16 else F32


def core_query_ts(c: int) -> list[int]:
    """Query t-indices owned by core c, in device processing order."""
    r = c % 4
    return [
        64 * (j - 1) + 16 * r + i for j in range(1, NJ + 1) for i in range(SUB)
    ]


def build_program(reps: int = 1, loop_iters: int = 0, stage: str = "full") -> bass.Bass:
    nc = bacc.Bacc()

    qt_d = nc.dram_tensor("qt", [H, 128], F32, kind="ExternalInput")
    kt_d = nc.dram_tensor("kt", [H, T], F32, kind="ExternalInput")
    kb_d = nc.dram_tensor("kb", [T, H], F32, kind="ExternalInput")
    wqt_d = nc.dram_tensor("wqt", [H, H], F32, kind="ExternalInput")
    wkt_d = nc.dram_tensor("wkt", [H, H], F32, kind="ExternalInput")
    g_d = nc.dram_tensor("g", [2, P, 2 * SUB - 1], EDT, kind="ExternalInput")
    mask_d = nc.dram_tensor("mask", [SUB, 64], F32, kind="ExternalInput")
    id_d = nc.dram_tensor("ident", [P, P], F32, kind="ExternalInput")

    alpha_d = nc.dram_tensor("alpha_part", [128, T], F32, kind="ExternalOutput")
    ctx_d = nc.dram_tensor("ctx_part", [128, H], F32, kind="ExternalOutput")

    with TileContext(nc) as tc:
        with (
            tc.tile_pool(name="const", bufs=1) as cp,
            tc.tile_pool(name="proj", bufs=1) as pp,
            tc.tile_pool(name="epool", bufs=3 if E_BF16 else 2) as ep,
            tc.tile_pool(name="pscore", bufs=4, space=bass.MemorySpace.PSUM) as pss,
            tc.tile_pool(name="soft", bufs=2) as sp,
            tc.tile_pool(name="pstr", bufs=2, space=bass.MemorySpace.PSUM) as pst,
            tc.tile_pool(name="psctx", bufs=2, space=bass.MemorySpace.PSUM) as psc,
        ):
            # ---- constants / inputs to SBUF
            wqt = [cp.tile([P, H], F32, tag=f"wqt{a}", name=f"wqt{a}") for a in range(2)]
            wkt = [cp.tile([P, H], F32, tag=f"wkt{a}", name=f"wkt{a}") for a in range(2)]
            qts = [cp.tile([P, 128], F32, tag=f"qt{a}", name=f"qt{a}") for a in range(2)]
            kts = [cp.tile([P, T], F32, tag=f"kt{a}", name=f"kt{a}") for a in range(2)]
            kbs = [cp.tile([P, H], F32, tag=f"kb{s}", name=f"kb{s}") for s in range(4)]
            gs = [cp.tile([P, 2 * SUB - 1], EDT, tag=f"g{a}", name=f"g{a}") for a in range(2)]
            maskt = cp.tile([SUB, 64], F32, tag="mask", name="maskt")
            ident = cp.tile([P, P], F32, tag="ident", name="ident")

            for a in range(2):
                nc.sync.dma_start(out=wqt[a], in_=wqt_d[a * P : (a + 1) * P, :])
                nc.sync.dma_start(out=qts[a], in_=qt_d[a * P : (a + 1) * P, :])
                nc.sync.dma_start(out=wkt[a], in_=wkt_d[a * P : (a + 1) * P, :])
                nc.sync.dma_start(out=kts[a], in_=kt_d[a * P : (a + 1) * P, :])
                nc.sync.dma_start(out=gs[a], in_=g_d[a])
            for s in range(4):
                nc.sync.dma_start(out=kbs[s], in_=kb_d[s * P : (s + 1) * P, :])
            nc.sync.dma_start(out=maskt, in_=mask_d[:, :])
            nc.sync.dma_start(out=ident, in_=id_d[:, :])

            # ---- projections: QpT/KpT = W @ X_sel.T, rows [128a:128a+128]
            qpt = [pp.tile([P, 128], F32, tag=f"qpt{a}", name=f"qpt{a}") for a in range(2)]
            kpt = [pp.tile([P, T], EDT, tag=f"kpt{a}", name=f"kpt{a}") for a in range(2)]
            for a in range(2):
                psq = pss.tile([P, 128], F32, tag="scores", name="psq")
                for bp in range(2):
                    nc.tensor.matmul(
                        psq,
                        wqt[bp][:, a * P : (a + 1) * P],
                        qts[bp],
                        start=(bp == 0),
                        stop=(bp == 1),
                    )
                nc.vector.tensor_copy(qpt[a], psq)
            for a in range(2):
                psk = pss.tile([P, T], F32, tag="scores", name="psk")
                for bp in range(2):
                    nc.tensor.matmul(
                        psk,
                        wkt[bp][:, a * P : (a + 1) * P],
                        kts[bp],
                        start=(bp == 0),
                        stop=(bp == 1),
                    )
                nc.vector.tensor_copy(kpt[a], psk)

            # ---- main loop
            if loop_iters:
                with tc.For_i(
                    0,
                    loop_iters,
                    1,
                    hint_engines=(
                        mybir.EngineType.DVE,
                        mybir.EngineType.PE,
                        mybir.EngineType.Activation,
                    ),
                    staggered_reset=True,
                ):
                    _main_loop(nc, ep, pss, sp, pst, psc, qpt, kpt, gs, maskt,
                               ident, kbs, alpha_d, ctx_d, stage)
            else:
                for _rep in range(reps):
                    _main_loop(nc, ep, pss, sp, pst, psc, qpt, kpt, gs, maskt,
                               ident, kbs, alpha_d, ctx_d, stage)

    nc.compile()
    return nc


def _main_loop(nc, ep, pss, sp, pst, psc, qpt, kpt, gs, maskt, ident, kbs,
               alpha_d, ctx_d, stage="full"):
    AF = mybir.ActivationFunctionType
    AX = mybir.AxisListType

    def front(j):
        """adds + tanh + score matmuls for width group j; returns psum tile."""
        w = 64 * j
        qb = SUB * (j - 1)
        # broadcast add: E[h, a, i, s] = Kp[h, s] + Qp[h, qb+i]
        # a-major so tanh(a=0) only waits on the first 16 adds
        e = ep.tile([P, 2, SUB, w], EDT, tag="e", name="e")
        for a in range(2):
            for i in range(SUB):
                nc.vector.tensor_scalar_add(
                    e[:, a, i, :],
                    kpt[a][:, :w],
                    qpt[a][:, qb + i : qb + i + 1],
                )
            if stage != "adds":
                nc.scalar.activation(e[:, a], e[:, a], AF.Tanh)
        if stage in ("tanh", "adds"):
            return None
        # scores[i, s] = sum_h v[h] * E[h, a, i, s]; a-major so the a=0
        # matmul sweep overlaps the a=1 tanh
        ps_s = pss.tile([SUB, T], F32, tag="scores", name="scores")
        for a in range(2):
            for i in range(SUB):
                nc.tensor.matmul(
                    ps_s[:, :w],
                    gs[a][:, SUB - 1 - i : 2 * SUB - 1 - i],
                    e[:, a, i, :],
                    start=(i == 0 and a == 0),
                    stop=(i == SUB - 1 and a == 1),
                )
        return ps_s

    def tail(j, ps_s):
        """softmax + ctx for width group j (emitted one group late so the
        exp never blocks the next group's tanh in ACT's FIFO)."""
        w = 64 * j
        qb = SUB * (j - 1)
        if stage == "scores":
            junk = sp.tile([SUB, T], F32, tag="junk", name="junk")
            nc.vector.tensor_copy(junk[:, :w], ps_s[:, :w])
            return
        # softmax over s; |scores| <= sum|v| ~ 15 so exp needs no max
        # subtraction. Additive causal mask pre-exp; exp emits the row-sum
        # via accum_out so DVE does no reduce.
        nc.vector.tensor_add(ps_s[:, w - 64 : w], ps_s[:, w - 64 : w], maskt)
        exps = sp.tile([SUB, T], F32, tag="expS", name="expS")
        ssum = sp.tile([SUB, 1], F32, tag="ssum", name="ssum")
        nc.scalar.activation(exps[:, :w], ps_s[:, :w], AF.Exp, accum_out=ssum)
        rsum = sp.tile([SUB, 1], F32, tag="rsum", name="rsum")
        nc.vector.reciprocal(rsum, ssum)

        alpha = sp.tile([SUB, T], F32, tag="alpha", name="alpha")
        if w < T:
            nc.gpsimd.memset(alpha[:, w:], 0.0)
        nc.vector.tensor_scalar_mul(alpha[:, :w], exps[:, :w], rsum)
        nc.sync.dma_start(out=alpha_d[qb : qb + SUB, :], in_=alpha)
        if stage == "softmax":
            return
        # ctx = alpha @ K_b = diag(1/sum) (exps @ K_b): transpose the
        # unnormalized exps in 128-wide chunks (zero-pad the tail for odd
        # j), matmul against 128-row K tiles, scale rows by 1/sum on the
        # way out of PSUM.
        nk = (j + 1) // 2
        if j % 2 == 1:
            nc.gpsimd.memset(exps[:, w : w + 64], 0.0)
        alphat = sp.tile([P, 4, SUB], F32, tag="alphat", name="alphat")
        for s in range(nk):
            ps_t = pst.tile([P, SUB], F32, tag="tr", name="tr")
            nc.tensor.transpose(
                ps_t, exps[:, s * P : (s + 1) * P], ident[:SUB, :SUB]
            )
            nc.vector.tensor_copy(alphat[:, s, :], ps_t)
        ps_c = psc.tile([SUB, H], F32, tag="ctx", name="ctx")
        for s in range(nk):
            nc.tensor.matmul(
                ps_c,
                alphat[:, s, :],
                kbs[s],
                start=(s == 0),
                stop=(s == nk - 1),
            )
        ctxs = sp.tile([SUB, H], F32, tag="ctxs", name="ctxs")
        nc.vector.tensor_scalar_mul(ctxs, ps_c, rsum)
        nc.sync.dma_start(out=ctx_d[qb : qb + SUB, :], in_=ctxs)

    pending = None
    for j in range(NJ, 0, -1):
        ps = front(j)
        if pending is not None:
            tail(pending[0], pending[1])
        pending = (j, ps) if ps is not None else None
    if pending is not None:
        tail(pending[0], pending[1])


def _to_edt(x):
    if E_BF16:
        import ml_dtypes

        return x.astype(ml_dtypes.bfloat16)
    return x


def make_in_maps(Q, K, Wq, Wk, v):
    """Per-core input dicts. All host-side numpy prep."""
    Q = np.ascontiguousarray(np.asarray(Q, dtype=np.float32))
    K = np.ascontiguousarray(np.asarray(K, dtype=np.float32))
    Wq = np.asarray(Wq, dtype=np.float32)
    Wk = np.asarray(Wk, dtype=np.float32)
    v = np.asarray(v, dtype=np.float32)

    wqt = np.ascontiguousarray(Wq.T)
    wkt = np.ascontiguousarray(Wk.T)
    g = np.zeros((2, P, 2 * SUB - 1), dtype=np.float32)
    g[0, :, SUB - 1] = v[:P]
    g[1, :, SUB - 1] = v[P:]
    g = _to_edt(g)
    ident = np.eye(P, dtype=np.float32)
    kbs = [K[b] for b in range(B)]

    in_maps = []
    for c in range(NCORES):
        b, r = c // 4, c % 4
        ts = core_query_ts(c)
        mask = np.zeros((SUB, 64), dtype=np.float32)
        for i in range(SUB):
            mask[i, 16 * r + i + 1 :] = NEG
        in_maps.append(
            {
                "qt": np.ascontiguousarray(Q[b][ts].T),
                "kt": np.ascontiguousarray(K[b].T),
                "kb": kbs[b],
                "wqt": wqt,
                "wkt": wkt,
                "g": g,
                "mask": mask,
                "ident": ident,
            }
        )
    return in_maps


_NC_CACHE: dict = {}


def get_program(reps: int = 1, loop_iters: int = 0, stage: str = "full") -> bass.Bass:
    key = f"nc{reps}_{loop_iters}_{stage}"
    if key not in _NC_CACHE:
        _NC_CACHE[key] = build_program(reps, loop_iters, stage)
    return _NC_CACHE[key]


def get_runner(reps: int = 1, loop_iters: int = 0, stage: str = "full"):
    """Compiled multi-core PJRT callable, cached. call(in_maps) -> per-core dicts."""
    key = f"run{reps}_{loop_iters}_{stage}"
    if key in _NC_CACHE:
        return _NC_CACHE[key]
    import jax
    import numpy as _np
    from jax.experimental.shard_map import shard_map
    from jax.sharding import Mesh, PartitionSpec
    from concourse import bass2jax

    bass2jax.install_neuronx_cc_hook()
    nc = get_program(reps, loop_iters, stage)
    assert nc.dbg_addr is None
    pname = nc.partition_id_tensor.name if nc.partition_id_tensor else None

    in_names, out_names, out_avals = [], [], []
    for alloc in nc.m.functions[0].allocations:
        if not isinstance(alloc, mybir.MemoryLocationSet):
            continue
        name = alloc.memorylocations[0].name
        if alloc.kind == "ExternalInput":
            if name != pname:
                in_names.append(name)
        elif alloc.kind == "ExternalOutput":
            out_names.append(name)
            out_avals.append(
                jax.core.ShapedArray(
                    tuple(alloc.tensor_shape), mybir.dt.np(alloc.dtype)
                )
            )
    n_params = len(in_names)
    n_outs = len(out_avals)
    all_names = in_names + out_names
    if pname is not None:
        all_names = all_names + [pname]
    all_names = tuple(all_names)

    def _body(*args):
        operands = list(args)
        if pname is not None:
            operands.append(bass2jax.partition_id_tensor())
        outs = bass2jax._bass_exec_p.bind(
            *operands,
            out_avals=tuple(out_avals),
            in_names=all_names,
            out_names=tuple(out_names),
            lowering_input_output_aliases=(),
            sim_require_finite=True,
            sim_require_nnan=True,
            nc=nc,
        )
        return tuple(outs)

    devices = jax.devices()[:NCORES]
    mesh = Mesh(np.asarray(devices), ("core",))
    in_specs = (PartitionSpec("core"),) * (n_params + n_outs)
    out_specs = (PartitionSpec("core"),) * n_outs
    donate = tuple(range(n_params, n_params + n_outs))
    sharded = jax.jit(
        shard_map(
            _body, mesh=mesh, in_specs=in_specs, out_specs=out_specs,
            check_rep=False,
        ),
        donate_argnums=donate,
        keep_unused=True,
    )

    def call(in_maps):
        concat_in = [
            _np.concatenate([_np.asarray(m[name]) for m in in_maps], axis=0)
            for name in in_names
        ]
        concat_zeros = [
            _np.zeros((NCORES * a.shape[0], *a.shape[1:]), a.dtype)
            for a in out_avals
        ]
        out_arrs = sharded(*concat_in, *concat_zeros)
        out_arrs = [_np.asarray(o) for o in out_arrs]
        return [
            {
                name: out_arrs[i].reshape(NCORES, *out_avals[i].shape)[c]
                for i, name in enumerate(out_names)
            }
            for c in range(NCORES)
        ]

    _NC_CACHE[key] = call
    return call


def run_spmd(Q, K, Wq, Wk, v, **kwargs):
    nc = get_program()
    in_maps = make_in_maps(Q, K, Wq, Wk, v)
    return run_bass_kernel_spmd(nc, in_maps, core_ids=list(range(NCORES)), **kwargs)


def assemble(results):
    ctx = np.zeros((B, T, H), dtype=np.float32)
    alpha = np.zeros((B, T, T), dtype=np.float32)
    for c in range(NCORES):
        b = c // 4
        ts = core_query_ts(c)
        ctx[b, ts, :] = results[c]["ctx_part"]
        alpha[b, ts, :] = results[c]["alpha_part"]
    return ctx, alpha


def kernel(Q, K, Wq, Wk, v):
    call = get_runner()
    results = call(make_in_maps(Q, K, Wq, Wk, v))
    return assemble(results)
